# revision 14
# baseline (speedup 1.0000x reference)
"""Parametrized Bass/Tile transformer-block kernel for TRN2, 8-core SPMD.

Sharding: core c -> (batch b=c//2, query parity p=c%2). Each core computes the
output rows for tokens {2t+p} of its batch element. Fully uniform program, no
collectives; causal structure is uniform because local q-block i attends
exactly 2i+2 key blocks on every core (parity handled by a fixed mask).

Layout convention: "transposed" activations [feature, token] so matmuls chain
without transposes; per-token LN2 stats via ones-matmul column sums; softmax
denominator via a fused ones-column in the AV matmul.

Schedule: the block is emitted as interleaved "units" so the Tile scheduler
can fill the PE during the scalar-engine-bound (exp) attention phase.
Attention superblocks mp=0,1 interleave with the tail of the K/V projections;
mp=2,3 interleave with Wo/LN2/FFN1 for the first query group; the second
query group plus FFN2 run straight-line at the end (PE-bound, no scalar
pressure).  Wo/W1/W2 are streamed in small column tiles so all long-lived
tensors fit in SBUF simultaneously (the pool allocator is a strict per-side
LIFO stack, so lifetimes are arranged stack-fashion).
"""
import sys
for _p in ("/opt/trn_rl_repo", "/root/.axon_site/_ro/trn_rl_repo"):
    if _p not in sys.path:
        sys.path.append(_p)

from collections import deque
from contextlib import ExitStack

import numpy as np
import ml_dtypes

import concourse.bass as bass
import concourse.mybir as mybir
import concourse.tile as tile
from concourse import bacc
from concourse.bass import ts, ds

AF = mybir.ActivationFunctionType
DT = mybir.dt
BF = ml_dtypes.bfloat16
F8 = mybir.dt.np(mybir.dt.float8e4)
DR = mybir.MatmulPerfMode.DoubleRow
P = 128
EPS = 1e-5
WSCALE = 32.0   # fp8 weight pre-scale (host) undone at psum->SBUF copy


def build_program(D, T, H, DK, FF, reps=1, pipe=True, tune=True):
    Tq = T // 2
    NCD = D // P            # feature chunks
    NCF = FF // P           # hidden chunks
    NHP = H // 2            # head pairs
    NQG = Tq // 512         # 512-token groups over own queries
    NM = Tq // 256          # attention q superblocks (256 wide)
    NKB = T // P            # key blocks
    WV = min(512, D)        # V-projection column group width
    NVG = D // WV
    HPG = WV // DK          # heads per V col group
    assert D == H * DK and Tq % 512 == 0 and NHP == NCD

    nc = bacc.Bacc(None, target_bir_lowering=False)

    NCP = NCD // 2          # fp8 DoubleRow chunk pairs

    # ---- DRAM I/O ----
    h1t_d = nc.dram_tensor("h1t", [D, T], DT.float8e4, kind="ExternalInput")
    h1qt_d = nc.dram_tensor("h1qt", [D, Tq], DT.float8e4, kind="ExternalInput")
    xqt_d = nc.dram_tensor("xqt", [D, Tq], DT.float32, kind="ExternalInput")
    wq_d = nc.dram_tensor("wq", [D, D], DT.float8e4, kind="ExternalInput")
    wk_d = nc.dram_tensor("wk", [D, D], DT.float8e4, kind="ExternalInput")
    wv_d = nc.dram_tensor("wv", [D, D], DT.float8e4, kind="ExternalInput")
    wo_d = nc.dram_tensor("wo", [D, D], DT.bfloat16, kind="ExternalInput")
    w1_d = nc.dram_tensor("w1", [D, FF], DT.bfloat16, kind="ExternalInput")
    w2_d = nc.dram_tensor("w2", [FF, D], DT.bfloat16, kind="ExternalInput")
    b1c_d = nc.dram_tensor("b1c", [P, NCF], DT.float32, kind="ExternalInput")
    mask_d = nc.dram_tensor("maskc", [P, 2, 2, 256], DT.bfloat16,
                            kind="ExternalInput")
    outt_d = nc.dram_tensor("outt", [D, Tq], DT.float32, kind="ExternalOutput")

    h1t_r = h1t_d[:].rearrange("(c p) t -> p c t", p=P)
    h1qt_r = h1qt_d[:].rearrange("(c p) t -> p c t", p=P)
    xqt_r = xqt_d[:].rearrange("(c p) t -> p c t", p=P)
    wq_r = wq_d[:].rearrange("(c p) n -> p c n", p=P)
    wk_r = wk_d[:].rearrange("(c p) n -> p c n", p=P)
    wv_r = wv_d[:].rearrange("(c p) n -> p c n", p=P)
    wo_r = wo_d[:].rearrange("(c p) n -> p c n", p=P)
    w1_r = w1_d[:].rearrange("(c p) n -> p c n", p=P)
    w2_r = w2_d[:].rearrange("(c p) n -> p c n", p=P)
    outt_r = outt_d[:].rearrange("(c p) t -> p c t", p=P)

    with tile.TileContext(nc) as tc, ExitStack() as top:
        constp = top.enter_context(tc.tile_pool(name="const", bufs=1))
        mask_sb = constp.tile([P, 2, 2, 256], DT.bfloat16)
        nc.sync.dma_start(mask_sb[:], mask_d[:])
        b1c_sb = constp.tile([P, NCF], DT.float32)
        nc.sync.dma_start(b1c_sb[:], b1c_d[:])
        ones_bf = constp.tile([P, 1], DT.bfloat16)
        nc.vector.memset(ones_bf[:], 1.0)
        ones_f = constp.tile([1, P], DT.float32)
        nc.vector.memset(ones_f[:], 1.0)
        ones_r = constp.tile([1, P], DT.float32r)
        with nc.allow_low_precision(reason="f32r ones"):
            nc.vector.tensor_copy(ones_r[:], ones_f[:])
        zero_b = constp.tile([P, 1], DT.float32)
        nc.vector.memset(zero_b[:], 0.0)
        eps_b = constp.tile([1, 1], DT.float32)
        nc.vector.memset(eps_b[:], EPS)

        for _rep in range(reps):
            # ------- left stack (LIFO): w_e | w_co | qkv | a2 | w_a | h1q -------
            we_cm = tc.tile_pool(name="w_e", bufs=3, side="left")
            w_e = we_cm.__enter__()
            wco_cm = tc.tile_pool(name="w_co", bufs=2, side="left")
            w_co = wco_cm.__enter__()

            qkv_cm = tc.tile_pool(name="p_qkv", bufs=1, side="left")
            p_qkv = qkv_cm.__enter__()
            qt_sb = p_qkv.tile([P, NHP, Tq], DT.bfloat16)
            kt_sb = p_qkv.tile([P, NHP, T], DT.bfloat16)
            v4_sb = p_qkv.tile([P, NKB, H, 65], DT.float8e4)

            a2_cm = tc.tile_pool(name="p_a2", bufs=1, side="left")
            p_a2 = a2_cm.__enter__()
            h1t_sb = p_a2.tile([P, NCD, T], DT.float8e4)
            wv_sb = p_a2.tile([P, NCD, D], DT.float8e4)

            wa_cm = tc.tile_pool(name="w_a", bufs=2, side="left")
            w_a = wa_cm.__enter__()

            h1q_cm = tc.tile_pool(name="p_h1q", bufs=1, side="left")
            p_h1q = h1q_cm.__enter__()
            h1qt_sb = p_h1q.tile([P, NCD, Tq], DT.float8e4)

            # ------- PSUM stack (LIFO): ps_c | ps_s | ps_o -------
            psc_cm = tc.tile_pool(name="ps_c", bufs=2, space="PSUM")
            ps_c = psc_cm.__enter__()
            pss_cm = tc.tile_pool(name="ps_s", bufs=2, space="PSUM")
            ps_s = pss_cm.__enter__()
            pso_cm = tc.tile_pool(name="ps_o", bufs=2, space="PSUM")
            ps_o = pso_cm.__enter__()

            # ---------------- input DMAs ----------------
            for c in range(NCD):
                nc.gpsimd.dma_start(h1qt_sb[:, c], h1qt_r[:, c])
            for c in range(NCD):
                nc.sync.dma_start(wv_sb[:, c], wv_r[:, c])
            for g in range(T // 512):
                for c in range(NCD):
                    eng = nc.scalar if (c % 2 == 0) else nc.gpsimd
                    eng.dma_start(h1t_sb[:, c, ds(512 * g, 512)],
                                  h1t_r[:, c, ds(512 * g, 512)])
            # softmax-denominator ones-column
            nc.vector.memset(v4_sb[:, :, :, 64:65], 1.0)

            # per-group tiles, created lazily at phase boundaries
            x1_t, xh_t, g1_t = {}, {}, {}

            # ---------------- unit emitters ----------------
            def q_unit(g, hp):
                wq_t = w_a.tile([P, NCD, P], DT.float8e4, tag="wq", name="wq_t")
                nc.sync.dma_start(wq_t[:], wq_r[:, :, ds(P * hp, P)])
                ps_q = ps_c.tile([P, 512], DT.float32, tag="chain", name="ps_q")
                for cp in range(NCP):
                    nc.tensor.matmul(
                        ps_q[:], wq_t[:, 2 * cp:2 * cp + 2, :],
                        h1qt_sb[:, 2 * cp:2 * cp + 2, ds(512 * g, 512)],
                        start=(cp == 0), stop=(cp == NCP - 1), perf_mode=DR)
                nc.vector.tensor_scalar_mul(
                    qt_sb[:, hp, ds(512 * g, 512)], ps_q[:], 1.0 / WSCALE)

            def k_unit(g, hp):
                wk_t = w_a.tile([P, NCD, P], DT.float8e4, tag="wk", name="wk_t")
                nc.sync.dma_start(wk_t[:], wk_r[:, :, ds(P * hp, P)])
                ps_k = ps_c.tile([P, 512], DT.float32, tag="chain", name="ps_k")
                for cp in range(NCP):
                    nc.tensor.matmul(
                        ps_k[:], wk_t[:, 2 * cp:2 * cp + 2, :],
                        h1t_sb[:, 2 * cp:2 * cp + 2, ds(512 * g, 512)],
                        start=(cp == 0), stop=(cp == NCP - 1), perf_mode=DR)
                nc.vector.tensor_scalar_mul(
                    kt_sb[:, hp, ds(512 * g, 512)], ps_k[:], 1.0 / WSCALE)

            def v_unit(tb, g2):
                ps_v = ps_c.tile([P, WV], DT.float32, tag="chain", name="ps_v")
                for cp in range(NCP):
                    nc.tensor.matmul(
                        ps_v[:], h1t_sb[:, 2 * cp:2 * cp + 2, ds(P * tb, P)],
                        wv_sb[:, 2 * cp:2 * cp + 2, ds(WV * g2, WV)],
                        start=(cp == 0), stop=(cp == NCP - 1), perf_mode=DR)
                for hh in range(HPG):
                    h = HPG * g2 + hh
                    nc.vector.tensor_scalar_mul(
                        v4_sb[:, tb, h, 0:64], ps_v[:, ds(64 * hh, 64)],
                        1.0 / WSCALE)

            def emit_av(po, hp, K0, exs, nkv):
                for hi in range(2):
                    h = 2 * hp + hi
                    for j in range(2):
                        kb = K0 + 2 * j
                        nc.tensor.matmul(
                            po[hi][0:65, :],
                            v4_sb[:, kb:kb + 2, h, 0:65],
                            exs[hi][:, j],
                            start=(kb == 0), stop=(kb + 2 == nkv),
                            perf_mode=DR)

            def attn_unit(mp, hp):
                # AV emission lags scores/exp by one key-block quad so the
                # PE always has independent scores work ahead of each
                # exp-blocked AV.
                nkv = 4 * mp + 4
                po = [ps_o.tile([P, 256], DT.float32, tag="po", name=f"po{_i}")
                      for _i in range(2)]
                lag = None
                for Kq in range(mp + 1):
                    K0 = 4 * Kq
                    exs = []
                    for hi in range(2):
                        pb = 64 * hi
                        ps = ps_s.tile([P, 1024], DT.float32, tag="ps", name="ps")
                        for dk in range(4):
                            nc.tensor.matmul(
                                ps[:, ds(256 * dk, 256)],
                                kt_sb[pb:pb + 64, hp, ds(P * (K0 + dk), P)],
                                qt_sb[pb:pb + 64, hp, ds(256 * mp, 256)],
                                start=True, stop=True,
                                tile_position=(pb, 0))
                        ex = p_exp.tile([P, 2, 2, 256], DT.float8e4, tag="ex",
                                        name="ex")
                        nc.scalar.activation(ex[:], ps[:], AF.Exp, bias=zero_b[:],
                                             scale=float(DK) ** -0.5)
                        if Kq == mp:
                            nc.vector.tensor_mul(ex[:], ex[:], mask_sb[:])
                        exs.append(ex)
                    if lag is not None:
                        emit_av(po, hp, lag[0], lag[1], nkv)
                    lag = (K0, exs)
                emit_av(po, hp, lag[0], lag[1], nkv)
                for hi in range(2):
                    rec = p_nrm.tile([1, 256], DT.float32r, tag="rec", name="rec")
                    with nc.allow_low_precision(reason="f32r broadcast operand"):
                        nc.vector.reciprocal(rec[:], po[hi][64:65, :])
                    pbc = ps_c.tile([P, 256], DT.float32, tag="chain", name="pbc")
                    nc.tensor.matmul(
                        pbc[0:64, :], ones_r[:, 0:64], rec[:],
                        start=True, stop=True)
                    rb = p_nrm.tile([64, 256], DT.float32, tag="rb", name="rb")
                    nc.vector.tensor_copy(rb[:], pbc[0:64, :])
                    nc.vector.tensor_mul(
                        ot_sb[64 * hi:64 * hi + 64, hp, ds(256 * mp, 256)],
                        po[hi][0:64, :], rb[:])

            def c_unit(g, c):
                wo_t = w_co.tile([P, NCD, P], DT.bfloat16, tag="wo", name="wo_t")
                nc.sync.dma_start(wo_t[:], wo_r[:, :, ds(P * c, P)])
                ps_x = ps_c.tile([P, 512], DT.float32, tag="chain", name="ps_x")
                for k in range(NCD):
                    nc.tensor.matmul(
                        ps_x[:], wo_t[:, k],
                        ot_sb[:, k, ds(512 * g, 512)],
                        start=(k == 0), stop=(k == NCD - 1))
                xq_t = p_xq.tile([P, 512], DT.float32, tag="xq", name="xq_t")
                nc.sync.dma_start(xq_t[:], xqt_r[:, c, ds(512 * g, 512)])
                nc.vector.tensor_add(x1_t[g][:, c, :], ps_x[:], xq_t[:])

            def d_unit(g):
                x1g = x1_t[g]
                ps1 = ps_c.tile([P, 512], DT.float32, tag="chain", name="ps1")
                for c in range(NCD):
                    nc.tensor.matmul(ps1[0:1, :], ones_bf[:], x1g[:, c, :],
                                     start=(c == 0), stop=(c == NCD - 1))
                ps2 = ps_c.tile([P, 512], DT.float32, tag="chain", name="ps2")
                for c in range(NCD):
                    sq = p_sq.tile([P, 512], DT.bfloat16, tag="sq", name="sq")
                    nc.vector.tensor_mul(sq[:], x1g[:, c, :], x1g[:, c, :])
                    nc.tensor.matmul(ps2[0:1, :], ones_bf[:], sq[:],
                                     start=(c == 0), stop=(c == NCD - 1))
                mu = p_st.tile([1, 512], DT.float32, tag="mu", name="mu")
                nc.vector.tensor_scalar_mul(mu[:], ps1[0:1, :], 1.0 / D)
                msq = p_st.tile([1, 512], DT.float32, tag="msq", name="msq")
                nc.vector.tensor_mul(msq[:], mu[:], mu[:])
                nc.vector.scalar_tensor_tensor(
                    msq[:], ps2[0:1, :], 1.0 / D, msq[:],
                    mybir.AluOpType.mult, mybir.AluOpType.subtract)
                sd = p_st.tile([1, 512], DT.float32, tag="sd", name="sd")
                nc.scalar.activation(sd[:], msq[:], AF.Sqrt, bias=eps_b[:])
                r2 = p_st.tile([1, 512], DT.float32r, tag="r2", name="r2")
                with nc.allow_low_precision(reason="f32r broadcast operand"):
                    nc.vector.reciprocal(r2[:], sd[:])
                m2r = p_st.tile([1, 512], DT.float32r, tag="m2r", name="m2r")
                with nc.allow_low_precision(reason="f32r broadcast operand"):
                    nc.vector.tensor_mul(m2r[:], mu[:], r2[:])
                pb1 = ps_c.tile([P, 512], DT.float32, tag="chain", name="pb1")
                nc.tensor.matmul(pb1[:], ones_r[:], r2[:], start=True, stop=True)
                r2b = p_sq.tile([P, 512], DT.float32, tag="r2b", name="r2b",
                                bufs=1)
                nc.vector.tensor_copy(r2b[:], pb1[:])
                pb2 = ps_c.tile([P, 512], DT.float32, tag="chain", name="pb2")
                nc.tensor.matmul(pb2[:], ones_r[:], m2r[:], start=True, stop=True)
                m2rb = p_sq.tile([P, 512], DT.float32, tag="m2rb", name="m2rb",
                                 bufs=1)
                nc.vector.tensor_copy(m2rb[:], pb2[:])
                for c in range(NCD):
                    tmp = p_sq.tile([P, 512], DT.float32, tag="tmp", name="tmp")
                    nc.vector.tensor_mul(tmp[:], x1g[:, c, :], r2b[:])
                    nc.vector.tensor_sub(xh_t[g][:, c, :], tmp[:], m2rb[:])

            def e_unit(g, m):
                w1_t = w_e.tile([P, NCD, P], DT.bfloat16, tag="w1", name="w1_t")
                nc.gpsimd.dma_start(w1_t[:], w1_r[:, :, ds(P * m, P)])
                ps_f = ps_c.tile([P, 512], DT.float32, tag="chain", name="ps_f")
                for c in range(NCD):
                    nc.tensor.matmul(
                        ps_f[:], w1_t[:, c], xh_t[g][:, c, :],
                        start=(c == 0), stop=(c == NCD - 1))
                nc.scalar.activation(
                    g1_t[g][:, m, :], ps_f[:], AF.Gelu, bias=b1c_sb[:, ds(m, 1)])

            def f_unit(g, c, w2c, ps_f):
                pf = ps_f.tile([P, 512], DT.float32, tag="pf", name="pf")
                for hh in range(NCF):
                    nc.tensor.matmul(
                        pf[:], w2c[:, hh], g1_t[g][:, hh, :],
                        start=(hh == 0), stop=(hh == NCF - 1))
                out_t = p_out.tile([P, 512], DT.float32, tag="out", name="out_t")
                nc.vector.tensor_add(out_t[:], pf[:], x1_t[g][:, c, :])
                nc.sync.dma_start(outt_r[:, c, ds(512 * g, 512)], out_t[:])

            # ---------------- emission schedule ----------------
            filler = deque()

            def drain(n):
                for _ in range(n):
                    if not filler:
                        return
                    filler.popleft()()

            # Phase 0a: q projections (the only h1qt consumers).
            for g in range(NQG):
                for hp in range(NHP):
                    q_unit(g, hp)
            h1q_cm.__exit__(None, None, None)

            # Phase 0b: minimum K/V work before attention mp=0 can start.
            for hp in range(NHP):
                k_unit(0, hp)
            for tb in range(4):
                for g2 in range(NVG):
                    v_unit(tb, g2)

            # Fillers for phase 1 (rest of K/V, dependency order).
            for hp in range(NHP):
                filler.append(lambda hp=hp: k_unit(1, hp))
            for tb in range(4, 8):
                for g2 in range(NVG):
                    filler.append(lambda tb=tb, g2=g2: v_unit(tb, g2))
            for hp in range(NHP):
                filler.append(lambda hp=hp: k_unit(2, hp))
            for tb in range(8, 12):
                for g2 in range(NVG):
                    filler.append(lambda tb=tb, g2=g2: v_unit(tb, g2))
            for hp in range(NHP):
                filler.append(lambda hp=hp: k_unit(3, hp))
            for tb in range(12, 16):
                for g2 in range(NVG):
                    filler.append(lambda tb=tb, g2=g2: v_unit(tb, g2))

            # attention output + exp scratch pools (right side)
            ot_cm = tc.tile_pool(name="p_ot", bufs=1, side="right")
            p_ot = ot_cm.__enter__()
            ot_sb = p_ot.tile([P, NCD, Tq], DT.bfloat16)
            pe_cm = tc.tile_pool(name="p_exp", bufs=6, side="right")
            p_exp = pe_cm.__enter__()
            pn_cm = tc.tile_pool(name="p_nrm", bufs=2, side="right")
            p_nrm = pn_cm.__enter__()

            # Phase 1: attention mp=0,1 interleaved with K/V tail.
            for mp in range(2):
                for hp in range(NHP):
                    attn_unit(mp, hp)
                    drain(2 if mp == 0 else 3)
            while filler:
                filler.popleft()()
            wa_cm.__exit__(None, None, None)
            a2_cm.__exit__(None, None, None)

            # phase-2 pools (right side, live to rep end)
            ph2_cm = tc.tile_pool(name="p_ph2", bufs=1, side="right")
            p_ph2 = ph2_cm.__enter__()
            x1_t[0] = p_ph2.tile([P, NCD, 512], DT.bfloat16, name="x1_0")
            xh_t[0] = p_ph2.tile([P, NCD, 512], DT.bfloat16, name="xh_0")
            g1_t[0] = p_ph2.tile([P, NCF, 512], DT.bfloat16, name="g1_0")
            pxq_cm = tc.tile_pool(name="p_xq", bufs=3, side="right")
            p_xq = pxq_cm.__enter__()
            psq_cm = tc.tile_pool(name="p_sq", bufs=2, side="right")
            p_sq = psq_cm.__enter__()
            pst_cm = tc.tile_pool(name="p_st", bufs=2, side="right")
            p_st = pst_cm.__enter__()

            # Fillers for phase 2: Wo+LN2+FFN1 for query group 0.
            for c in range(NCD):
                filler.append(lambda c=c: c_unit(0, c))
            filler.append(lambda: d_unit(0))
            for m in range(NCF):
                filler.append(lambda m=m: e_unit(0, m))

            # Phase 2: attention mp=2,3 interleaved with group-0 FFN work.
            for mp in range(2, 4):
                for hp in range(NHP):
                    attn_unit(mp, hp)
                    drain(3)
            while filler:
                filler.popleft()()

            # attention done: free scores/AV PSUM and q/k/v SBUF.
            pso_cm.__exit__(None, None, None)
            pss_cm.__exit__(None, None, None)
            qkv_cm.__exit__(None, None, None)

            # phase-3 pools
            ph3_cm = tc.tile_pool(name="p_ph3", bufs=1, side="right")
            p_ph3 = ph3_cm.__enter__()
            x1_t[1] = p_ph3.tile([P, NCD, 512], DT.bfloat16, name="x1_1")
            xh_t[1] = p_ph3.tile([P, NCD, 512], DT.bfloat16, name="xh_1")
            g1_t[1] = p_ph3.tile([P, NCF, 512], DT.bfloat16, name="g1_1")
            po_cm = tc.tile_pool(name="p_out", bufs=4, side="right")
            p_out = po_cm.__enter__()
            w2s_cm = tc.tile_pool(name="w2s", bufs=2, side="left")
            w2s = w2s_cm.__enter__()
            psf_cm = tc.tile_pool(name="ps_f", bufs=3, space="PSUM")
            ps_f = psf_cm.__enter__()

            # Phase 3: group-1 Wo/LN2/FFN1, then FFN2 for both groups with
            # w2 streamed per output column chunk.
            for c in range(NCD):
                c_unit(1, c)
            d_unit(1)
            for m in range(NCF):
                e_unit(1, m)
            for c in range(NCD):
                w2c = w2s.tile([P, NCF, P], DT.bfloat16, tag="w2c", name="w2c")
                nc.gpsimd.dma_start(w2c[:], w2_r[:, :, ds(P * c, P)])
                for g in range(NQG):
                    f_unit(g, c, w2c, ps_f)

            # rep teardown (LIFO per side)
            psf_cm.__exit__(None, None, None)
            w2s_cm.__exit__(None, None, None)
            po_cm.__exit__(None, None, None)
            ph3_cm.__exit__(None, None, None)
            pst_cm.__exit__(None, None, None)
            psq_cm.__exit__(None, None, None)
            pxq_cm.__exit__(None, None, None)
            ph2_cm.__exit__(None, None, None)
            pn_cm.__exit__(None, None, None)
            pe_cm.__exit__(None, None, None)
            ot_cm.__exit__(None, None, None)
            psc_cm.__exit__(None, None, None)
            wco_cm.__exit__(None, None, None)
            we_cm.__exit__(None, None, None)

    nc.compile()
    return nc


# ---------------- host side ----------------

def host_prep(x, Wq, Wk, Wv, Wo, bo, W1, b1, W2, b2, g1, be1, g2, be2):
    D = x.shape[2]
    H = Wq.shape[0]
    FF = W1.shape[1]
    NCF = FF // P
    f32 = np.float32

    mu = x.mean(-1, keepdims=True)
    var = ((x - mu) ** 2).mean(-1, keepdims=True)
    h1 = ((x - mu) / np.sqrt(var + EPS) * g1 + be1).astype(f32)

    shared = dict(
        wq=np.ascontiguousarray(
            WSCALE * Wq.transpose(1, 0, 2).reshape(D, -1)).astype(F8),
        wk=np.ascontiguousarray(
            WSCALE * Wk.transpose(1, 0, 2).reshape(D, -1)).astype(F8),
        wv=np.ascontiguousarray(
            WSCALE * Wv.transpose(1, 0, 2).reshape(D, -1)).astype(F8),
        wo=np.ascontiguousarray(Wo).astype(BF),
        w1=np.ascontiguousarray(g2[:, None] * W1).astype(BF),
        w2=np.ascontiguousarray(W2).astype(BF),
        b1c=np.ascontiguousarray((b1 + be2 @ W1).astype(f32).reshape(NCF, P).T),
    )
    per_core = []
    for c in range(8):
        b, p = c // 2, c % 2
        r = np.arange(P)[:, None]
        j = np.arange(256)[None, :]
        qoff = np.where(j < P, 2 * j + p, 256 + 2 * (j - P) + p)
        m4 = np.zeros((P, 4, 256), f32)
        for t in range(4):
            m4[:, t, :] = (P * t + r <= qoff)
        m = m4.reshape(P, 2, 2, 256)  # key-block-quad (pair, dk) layout
        per_core.append(dict(
            h1t=np.ascontiguousarray(h1[b].T).astype(F8),
            h1qt=np.ascontiguousarray(h1[b, p::2, :].T).astype(F8),
            xqt=np.ascontiguousarray(
                x[b, p::2, :].T + bo[:, None] + b2[:, None]).astype(f32),
            maskc=m.astype(BF),
            **shared,
        ))
    return per_core


def assemble(outts, B, T, D):
    out = np.zeros((B, T, D), np.float32)
    for c in range(8):
        b, p = c // 2, c % 2
        out[b, p::2, :] = outts[c].T
    return out


# ======================== top-level kernel entry ========================

_CACHE = {}


def _get_program():
    if "nc" not in _CACHE:
        _CACHE["nc"] = build_program(1024, 2048, 16, 64, 4096)
    return _CACHE["nc"]


def kernel(**inputs):
    """Full transformer block on 8 TRN2 NeuronCores.

    Takes the full unsharded inputs (as produced by setup_inputs()), shards
    (batch x query-parity) across 8 cores, runs the Bass SPMD kernel, and
    reassembles the full [4, 2048, 1024] float32 output.
    """
    from concourse.bass_utils import run_bass_kernel_spmd

    np_inputs = {k: np.asarray(v, np.float32) for k, v in inputs.items()}
    B, T, D = np_inputs["x"].shape
    nc = _get_program()
    per_core = host_prep(**np_inputs)
    res = run_bass_kernel_spmd(nc, per_core, list(range(8)))
    outts = [res.results[c]["outt"] for c in range(8)]
    return assemble(outts, B, T, D)


# revision 19
# speedup vs baseline: 1.4107x; 1.4107x over previous
"""Parametrized Bass/Tile transformer-block kernel for TRN2, 8-core SPMD.

Sharding: core c -> (batch b=c//2, query parity p=c%2). Each core computes the
output rows for tokens {2t+p} of its batch element. Fully uniform program, no
collectives; causal structure is uniform because local q-block i attends
exactly 2i+2 key blocks on every core (parity handled by a fixed mask).

Layout convention: "transposed" activations [feature, token] so matmuls chain
without transposes; per-token LN2 stats via ones-matmul column sums; softmax
denominator via a fused ones-column in the AV matmul.

Schedule: the block is emitted as interleaved "units" so the Tile scheduler
can fill the PE during the scalar-engine-bound (exp) attention phase.
Attention superblocks mp=0,1 interleave with the tail of the K/V projections;
mp=2,3 interleave with Wo/LN2/FFN1 for the first query group; the second
query group plus FFN2 run straight-line at the end (PE-bound, no scalar
pressure).  Wo/W1/W2 are streamed in small column tiles so all long-lived
tensors fit in SBUF simultaneously (the pool allocator is a strict per-side
LIFO stack, so lifetimes are arranged stack-fashion).
"""
import sys
for _p in ("/opt/trn_rl_repo", "/root/.axon_site/_ro/trn_rl_repo"):
    if _p not in sys.path:
        sys.path.append(_p)

from collections import deque
from contextlib import ExitStack

import numpy as np
import ml_dtypes

import concourse.bass as bass
import concourse.mybir as mybir
import concourse.tile as tile
from concourse import bacc
from concourse.bass import ts, ds

AF = mybir.ActivationFunctionType
DT = mybir.dt
BF = ml_dtypes.bfloat16
F8 = mybir.dt.np(mybir.dt.float8e4)
DR = mybir.MatmulPerfMode.DoubleRow
P = 128
EPS = 1e-5
WSCALE = 32.0   # fp8 weight pre-scale (host) undone at psum->SBUF copy


def build_program(D, T, H, DK, FF, reps=1, pipe=True, tune=True):
    Tq = T // 2
    NCD = D // P            # feature chunks
    NCF = FF // P           # hidden chunks
    NHP = H // 2            # head pairs
    NQG = Tq // 512         # 512-token groups over own queries
    NM = Tq // 256          # attention q superblocks (256 wide)
    NKB = T // P            # key blocks
    WV = min(512, D)        # V-projection column group width
    NVG = D // WV
    HPG = WV // DK          # heads per V col group
    assert D == H * DK and Tq % 512 == 0 and NHP == NCD

    nc = bacc.Bacc(None, target_bir_lowering=False)

    NCP = NCD // 2          # fp8 DoubleRow chunk pairs

    # ---- DRAM I/O ----
    h1t_d = nc.dram_tensor("h1t", [D, T], DT.float8e4, kind="ExternalInput")
    h1qt_d = nc.dram_tensor("h1qt", [D, Tq], DT.float8e4, kind="ExternalInput")
    xqt_d = nc.dram_tensor("xqt", [D, Tq], DT.float32, kind="ExternalInput")
    wq_d = nc.dram_tensor("wq", [D, D], DT.float8e4, kind="ExternalInput")
    wk_d = nc.dram_tensor("wk", [D, D], DT.float8e4, kind="ExternalInput")
    wv_d = nc.dram_tensor("wv", [D, D], DT.float8e4, kind="ExternalInput")
    wo_d = nc.dram_tensor("wo", [D, D], DT.bfloat16, kind="ExternalInput")
    w1_d = nc.dram_tensor("w1", [D, FF], DT.bfloat16, kind="ExternalInput")
    w2_d = nc.dram_tensor("w2", [FF, D], DT.bfloat16, kind="ExternalInput")
    b1c_d = nc.dram_tensor("b1c", [P, NCF], DT.float32, kind="ExternalInput")
    mask_d = nc.dram_tensor("maskc", [P, 2, 2, 256], DT.bfloat16,
                            kind="ExternalInput")
    outt_d = nc.dram_tensor("outt", [D, Tq], DT.float32, kind="ExternalOutput")

    h1t_r = h1t_d[:].rearrange("(c p) t -> p c t", p=P)
    h1qt_r = h1qt_d[:].rearrange("(c p) t -> p c t", p=P)
    xqt_r = xqt_d[:].rearrange("(c p) t -> p c t", p=P)
    wq_r = wq_d[:].rearrange("(c p) n -> p c n", p=P)
    wk_r = wk_d[:].rearrange("(c p) n -> p c n", p=P)
    wv_r = wv_d[:].rearrange("(c p) n -> p c n", p=P)
    wo_r = wo_d[:].rearrange("(c p) n -> p c n", p=P)
    w1_r = w1_d[:].rearrange("(c p) n -> p c n", p=P)
    w2_r = w2_d[:].rearrange("(c p) n -> p c n", p=P)
    outt_r = outt_d[:].rearrange("(c p) t -> p c t", p=P)

    with tile.TileContext(nc) as tc, ExitStack() as top:
        constp = top.enter_context(tc.tile_pool(name="const", bufs=1))
        mask_sb = constp.tile([P, 2, 2, 256], DT.bfloat16)
        nc.sync.dma_start(mask_sb[:], mask_d[:])
        b1c_sb = constp.tile([P, NCF], DT.float32)
        nc.sync.dma_start(b1c_sb[:], b1c_d[:])
        ones_bf = constp.tile([P, 1], DT.bfloat16)
        nc.vector.memset(ones_bf[:], 1.0)
        ones_f = constp.tile([1, P], DT.float32)
        nc.vector.memset(ones_f[:], 1.0)
        ones_r = constp.tile([1, P], DT.float32r)
        with nc.allow_low_precision(reason="f32r ones"):
            nc.vector.tensor_copy(ones_r[:], ones_f[:])
        zero_b = constp.tile([P, 1], DT.float32)
        nc.vector.memset(zero_b[:], 0.0)
        eps_b = constp.tile([1, 1], DT.float32)
        nc.vector.memset(eps_b[:], EPS)

        for _rep in range(reps):
            # ------- left stack (LIFO): w_e | w_co | qkv | a2 | w_a | h1q -------
            we_cm = tc.tile_pool(name="w_e", bufs=3, side="left")
            w_e = we_cm.__enter__()
            wco_cm = tc.tile_pool(name="w_co", bufs=2, side="left")
            w_co = wco_cm.__enter__()

            qkv_cm = tc.tile_pool(name="p_qkv", bufs=1, side="left")
            p_qkv = qkv_cm.__enter__()
            qt_sb = p_qkv.tile([P, NHP, Tq], DT.bfloat16)
            kt_sb = p_qkv.tile([P, NHP, T], DT.bfloat16)
            v4_sb = p_qkv.tile([P, NKB, H, 65], DT.bfloat16)

            a2_cm = tc.tile_pool(name="p_a2", bufs=1, side="left")
            p_a2 = a2_cm.__enter__()
            h1t_sb = p_a2.tile([P, NCD, T], DT.float8e4)
            wv_sb = p_a2.tile([P, NCD, D], DT.float8e4)

            wa_cm = tc.tile_pool(name="w_a", bufs=2, side="left")
            w_a = wa_cm.__enter__()

            h1q_cm = tc.tile_pool(name="p_h1q", bufs=1, side="left")
            p_h1q = h1q_cm.__enter__()
            h1qt_sb = p_h1q.tile([P, NCD, Tq], DT.float8e4)

            # ------- PSUM stack (LIFO): ps_c | ps_s | ps_o -------
            psc_cm = tc.tile_pool(name="ps_c", bufs=2, space="PSUM")
            ps_c = psc_cm.__enter__()
            pss_cm = tc.tile_pool(name="ps_s", bufs=2, space="PSUM")
            ps_s = pss_cm.__enter__()
            pso_cm = tc.tile_pool(name="ps_o", bufs=2, space="PSUM")
            ps_o = pso_cm.__enter__()

            # ---------------- input DMAs ----------------
            for c in range(NCD):
                nc.gpsimd.dma_start(h1qt_sb[:, c], h1qt_r[:, c])
            for c in range(NCD):
                nc.sync.dma_start(wv_sb[:, c], wv_r[:, c])
            for g in range(T // 512):
                for c in range(NCD):
                    eng = nc.scalar if (c % 2 == 0) else nc.gpsimd
                    eng.dma_start(h1t_sb[:, c, ds(512 * g, 512)],
                                  h1t_r[:, c, ds(512 * g, 512)])
            # softmax-denominator ones-column
            nc.vector.memset(v4_sb[:, :, :, 64:65], 1.0)

            # per-group tiles, created lazily at phase boundaries
            x1_t, xh_t, g1_t = {}, {}, {}

            # ---------------- unit emitters ----------------
            def q_unit(g, hp):
                wq_t = w_a.tile([P, NCD, P], DT.float8e4, tag="wq", name="wq_t")
                nc.sync.dma_start(wq_t[:], wq_r[:, :, ds(P * hp, P)])
                ps_q = ps_c.tile([P, 512], DT.float32, tag="chain", name="ps_q")
                for cp in range(NCP):
                    nc.tensor.matmul(
                        ps_q[:], wq_t[:, 2 * cp:2 * cp + 2, :],
                        h1qt_sb[:, 2 * cp:2 * cp + 2, ds(512 * g, 512)],
                        start=(cp == 0), stop=(cp == NCP - 1), perf_mode=DR)
                nc.vector.tensor_scalar_mul(
                    qt_sb[:, hp, ds(512 * g, 512)], ps_q[:], 1.0 / WSCALE)

            def k_unit(g, hp):
                wk_t = w_a.tile([P, NCD, P], DT.float8e4, tag="wk", name="wk_t")
                nc.sync.dma_start(wk_t[:], wk_r[:, :, ds(P * hp, P)])
                ps_k = ps_c.tile([P, 512], DT.float32, tag="chain", name="ps_k")
                for cp in range(NCP):
                    nc.tensor.matmul(
                        ps_k[:], wk_t[:, 2 * cp:2 * cp + 2, :],
                        h1t_sb[:, 2 * cp:2 * cp + 2, ds(512 * g, 512)],
                        start=(cp == 0), stop=(cp == NCP - 1), perf_mode=DR)
                nc.vector.tensor_scalar_mul(
                    kt_sb[:, hp, ds(512 * g, 512)], ps_k[:], 1.0 / WSCALE)

            def v_unit(tb, g2):
                ps_v = ps_c.tile([P, WV], DT.float32, tag="chain", name="ps_v")
                for cp in range(NCP):
                    nc.tensor.matmul(
                        ps_v[:], h1t_sb[:, 2 * cp:2 * cp + 2, ds(P * tb, P)],
                        wv_sb[:, 2 * cp:2 * cp + 2, ds(WV * g2, WV)],
                        start=(cp == 0), stop=(cp == NCP - 1), perf_mode=DR)
                for hh in range(HPG):
                    h = HPG * g2 + hh
                    nc.vector.tensor_scalar_mul(
                        v4_sb[:, tb, h, 0:64], ps_v[:, ds(64 * hh, 64)],
                        1.0 / WSCALE)

            def emit_av(po, hp, K0, exs, nkv):
                for hi in range(2):
                    h = 2 * hp + hi
                    for dk in range(4):
                        kb = K0 + dk
                        nc.tensor.matmul(
                            po[hi][0:65, :],
                            v4_sb[:, kb, h, 0:65],
                            exs[hi][:, dk // 2, dk % 2],
                            start=(kb == 0), stop=(kb == nkv - 1))

            def attn_unit(mp, hp):
                # AV emission lags scores/exp by one key-block quad so the
                # PE always has independent scores work ahead of each
                # exp-blocked AV.
                nkv = 4 * mp + 4
                po = [ps_o.tile([P, 256], DT.float32, tag="po", name=f"po{_i}")
                      for _i in range(2)]
                lag = None
                for Kq in range(mp + 1):
                    K0 = 4 * Kq
                    exs = []
                    for hi in range(2):
                        pb = 64 * hi
                        ps = ps_s.tile([P, 1024], DT.float32, tag="ps", name="ps")
                        for dk in range(4):
                            nc.tensor.matmul(
                                ps[:, ds(256 * dk, 256)],
                                kt_sb[pb:pb + 64, hp, ds(P * (K0 + dk), P)],
                                qt_sb[pb:pb + 64, hp, ds(256 * mp, 256)],
                                start=True, stop=True,
                                tile_position=(pb, 0))
                        ex = p_exp.tile([P, 2, 2, 256], DT.bfloat16, tag="ex",
                                        name="ex")
                        nc.scalar.activation(ex[:], ps[:], AF.Exp, bias=zero_b[:],
                                             scale=float(DK) ** -0.5)
                        if Kq == mp:
                            nc.vector.tensor_mul(ex[:], ex[:], mask_sb[:])
                        exs.append(ex)
                    if lag is not None:
                        emit_av(po, hp, lag[0], lag[1], nkv)
                    lag = (K0, exs)
                emit_av(po, hp, lag[0], lag[1], nkv)
                for hi in range(2):
                    rec = p_nrm.tile([1, 256], DT.float32r, tag="rec", name="rec")
                    with nc.allow_low_precision(reason="f32r broadcast operand"):
                        nc.vector.reciprocal(rec[:], po[hi][64:65, :])
                    pbc = ps_c.tile([P, 256], DT.float32, tag="chain", name="pbc")
                    nc.tensor.matmul(
                        pbc[0:64, :], ones_r[:, 0:64], rec[:],
                        start=True, stop=True)
                    rb = p_nrm.tile([64, 256], DT.float32, tag="rb", name="rb")
                    nc.vector.tensor_copy(rb[:], pbc[0:64, :])
                    nc.vector.tensor_mul(
                        ot_sb[64 * hi:64 * hi + 64, hp, ds(256 * mp, 256)],
                        po[hi][0:64, :], rb[:])

            def c_unit(g, c):
                wo_t = w_co.tile([P, NCD, P], DT.bfloat16, tag="wo", name="wo_t")
                nc.sync.dma_start(wo_t[:], wo_r[:, :, ds(P * c, P)])
                ps_x = ps_c.tile([P, 512], DT.float32, tag="chain", name="ps_x")
                for k in range(NCD):
                    nc.tensor.matmul(
                        ps_x[:], wo_t[:, k],
                        ot_sb[:, k, ds(512 * g, 512)],
                        start=(k == 0), stop=(k == NCD - 1))
                xq_t = p_xq.tile([P, 512], DT.float32, tag="xq", name="xq_t")
                nc.sync.dma_start(xq_t[:], xqt_r[:, c, ds(512 * g, 512)])
                nc.vector.tensor_add(x1_t[g][:, c, :], ps_x[:], xq_t[:])

            def d_unit(g):
                x1g = x1_t[g]
                ps1 = ps_c.tile([P, 512], DT.float32, tag="chain", name="ps1")
                for c in range(NCD):
                    nc.tensor.matmul(ps1[0:1, :], ones_bf[:], x1g[:, c, :],
                                     start=(c == 0), stop=(c == NCD - 1))
                ps2 = ps_c.tile([P, 512], DT.float32, tag="chain", name="ps2")
                for c in range(NCD):
                    sq = p_sq.tile([P, 512], DT.bfloat16, tag="sq", name="sq")
                    nc.vector.tensor_mul(sq[:], x1g[:, c, :], x1g[:, c, :])
                    nc.tensor.matmul(ps2[0:1, :], ones_bf[:], sq[:],
                                     start=(c == 0), stop=(c == NCD - 1))
                mu = p_st.tile([1, 512], DT.float32, tag="mu", name="mu")
                nc.vector.tensor_scalar_mul(mu[:], ps1[0:1, :], 1.0 / D)
                msq = p_st.tile([1, 512], DT.float32, tag="msq", name="msq")
                nc.vector.tensor_mul(msq[:], mu[:], mu[:])
                nc.vector.scalar_tensor_tensor(
                    msq[:], ps2[0:1, :], 1.0 / D, msq[:],
                    mybir.AluOpType.mult, mybir.AluOpType.subtract)
                sd = p_st.tile([1, 512], DT.float32, tag="sd", name="sd")
                nc.scalar.activation(sd[:], msq[:], AF.Sqrt, bias=eps_b[:])
                r2 = p_st.tile([1, 512], DT.float32r, tag="r2", name="r2")
                with nc.allow_low_precision(reason="f32r broadcast operand"):
                    nc.vector.reciprocal(r2[:], sd[:])
                m2r = p_st.tile([1, 512], DT.float32r, tag="m2r", name="m2r")
                with nc.allow_low_precision(reason="f32r broadcast operand"):
                    nc.vector.tensor_mul(m2r[:], mu[:], r2[:])
                pb1 = ps_c.tile([P, 512], DT.float32, tag="chain", name="pb1")
                nc.tensor.matmul(pb1[:], ones_r[:], r2[:], start=True, stop=True)
                r2b = p_sq.tile([P, 512], DT.float32, tag="r2b", name="r2b",
                                bufs=1)
                nc.vector.tensor_copy(r2b[:], pb1[:])
                pb2 = ps_c.tile([P, 512], DT.float32, tag="chain", name="pb2")
                nc.tensor.matmul(pb2[:], ones_r[:], m2r[:], start=True, stop=True)
                m2rb = p_sq.tile([P, 512], DT.float32, tag="m2rb", name="m2rb",
                                 bufs=1)
                nc.vector.tensor_copy(m2rb[:], pb2[:])
                for c in range(NCD):
                    tmp = p_sq.tile([P, 512], DT.float32, tag="tmp", name="tmp")
                    nc.vector.tensor_mul(tmp[:], x1g[:, c, :], r2b[:])
                    nc.vector.tensor_sub(xh_t[g][:, c, :], tmp[:], m2rb[:])

            def e_unit(g, m):
                w1_t = w_e.tile([P, NCD, P], DT.bfloat16, tag="w1", name="w1_t")
                nc.gpsimd.dma_start(w1_t[:], w1_r[:, :, ds(P * m, P)])
                ps_f = ps_c.tile([P, 512], DT.float32, tag="chain", name="ps_f")
                for c in range(NCD):
                    nc.tensor.matmul(
                        ps_f[:], w1_t[:, c], xh_t[g][:, c, :],
                        start=(c == 0), stop=(c == NCD - 1))
                nc.scalar.activation(
                    g1_t[g][:, m, :], ps_f[:], AF.Gelu, bias=b1c_sb[:, ds(m, 1)])

            def f_unit(g, c, w2c, ps_f):
                pf = ps_f.tile([P, 512], DT.float32, tag="pf", name="pf")
                for hh in range(NCF):
                    nc.tensor.matmul(
                        pf[:], w2c[:, hh], g1_t[g][:, hh, :],
                        start=(hh == 0), stop=(hh == NCF - 1))
                out_t = p_out.tile([P, 512], DT.float32, tag="out", name="out_t")
                nc.vector.tensor_add(out_t[:], pf[:], x1_t[g][:, c, :])
                nc.sync.dma_start(outt_r[:, c, ds(512 * g, 512)], out_t[:])

            # ---------------- emission schedule ----------------
            filler = deque()

            def drain(n):
                for _ in range(n):
                    if not filler:
                        return
                    filler.popleft()()

            # Phase 0a: q projections (the only h1qt consumers).
            for g in range(NQG):
                for hp in range(NHP):
                    q_unit(g, hp)
            h1q_cm.__exit__(None, None, None)

            # Phase 0b: minimum K/V work before attention mp=0 can start.
            for hp in range(NHP):
                k_unit(0, hp)
            for tb in range(4):
                for g2 in range(NVG):
                    v_unit(tb, g2)

            # Fillers for phase 1 (rest of K/V, dependency order).
            for hp in range(NHP):
                filler.append(lambda hp=hp: k_unit(1, hp))
            for tb in range(4, 8):
                for g2 in range(NVG):
                    filler.append(lambda tb=tb, g2=g2: v_unit(tb, g2))
            for hp in range(NHP):
                filler.append(lambda hp=hp: k_unit(2, hp))
            for tb in range(8, 12):
                for g2 in range(NVG):
                    filler.append(lambda tb=tb, g2=g2: v_unit(tb, g2))
            for hp in range(NHP):
                filler.append(lambda hp=hp: k_unit(3, hp))
            for tb in range(12, 16):
                for g2 in range(NVG):
                    filler.append(lambda tb=tb, g2=g2: v_unit(tb, g2))

            # attention output + exp scratch pools (right side)
            ot_cm = tc.tile_pool(name="p_ot", bufs=1, side="right")
            p_ot = ot_cm.__enter__()
            ot_sb = p_ot.tile([P, NCD, Tq], DT.bfloat16)
            pe_cm = tc.tile_pool(name="p_exp", bufs=5, side="right")
            p_exp = pe_cm.__enter__()
            pn_cm = tc.tile_pool(name="p_nrm", bufs=2, side="right")
            p_nrm = pn_cm.__enter__()

            # Phase 1: attention mp=0,1 interleaved with K/V tail.
            for mp in range(2):
                for hp in range(NHP):
                    attn_unit(mp, hp)
                    drain(2 if mp == 0 else 3)
            while filler:
                filler.popleft()()
            wa_cm.__exit__(None, None, None)
            a2_cm.__exit__(None, None, None)

            # phase-2 pools (right side, live to rep end)
            ph2_cm = tc.tile_pool(name="p_ph2", bufs=1, side="right")
            p_ph2 = ph2_cm.__enter__()
            x1_t[0] = p_ph2.tile([P, NCD, 512], DT.bfloat16, name="x1_0")
            xh_t[0] = p_ph2.tile([P, NCD, 512], DT.bfloat16, name="xh_0")
            g1_t[0] = p_ph2.tile([P, NCF, 512], DT.bfloat16, name="g1_0")
            pxq_cm = tc.tile_pool(name="p_xq", bufs=3, side="right")
            p_xq = pxq_cm.__enter__()
            psq_cm = tc.tile_pool(name="p_sq", bufs=2, side="right")
            p_sq = psq_cm.__enter__()
            pst_cm = tc.tile_pool(name="p_st", bufs=1, side="right")
            p_st = pst_cm.__enter__()

            # Fillers for phase 2: Wo+LN2+FFN1 for query group 0.
            for c in range(NCD):
                filler.append(lambda c=c: c_unit(0, c))
            filler.append(lambda: d_unit(0))
            for m in range(NCF):
                filler.append(lambda m=m: e_unit(0, m))

            # Phase 2: attention mp=2,3 interleaved with group-0 FFN work.
            for mp in range(2, 4):
                for hp in range(NHP):
                    attn_unit(mp, hp)
                    drain(3)
            while filler:
                filler.popleft()()

            # attention done: free scores/AV PSUM and q/k/v SBUF.
            pso_cm.__exit__(None, None, None)
            pss_cm.__exit__(None, None, None)
            qkv_cm.__exit__(None, None, None)

            # phase-3 pools
            ph3_cm = tc.tile_pool(name="p_ph3", bufs=1, side="right")
            p_ph3 = ph3_cm.__enter__()
            x1_t[1] = p_ph3.tile([P, NCD, 512], DT.bfloat16, name="x1_1")
            xh_t[1] = p_ph3.tile([P, NCD, 512], DT.bfloat16, name="xh_1")
            g1_t[1] = p_ph3.tile([P, NCF, 512], DT.bfloat16, name="g1_1")
            po_cm = tc.tile_pool(name="p_out", bufs=4, side="right")
            p_out = po_cm.__enter__()
            w2s_cm = tc.tile_pool(name="w2s", bufs=2, side="left")
            w2s = w2s_cm.__enter__()
            psf_cm = tc.tile_pool(name="ps_f", bufs=3, space="PSUM")
            ps_f = psf_cm.__enter__()

            # Phase 3: group-1 Wo/LN2/FFN1, then FFN2 for both groups with
            # w2 streamed per output column chunk.
            for c in range(NCD):
                c_unit(1, c)
            d_unit(1)
            for m in range(NCF):
                e_unit(1, m)
            for c in range(NCD):
                w2c = w2s.tile([P, NCF, P], DT.bfloat16, tag="w2c", name="w2c")
                nc.gpsimd.dma_start(w2c[:], w2_r[:, :, ds(P * c, P)])
                for g in range(NQG):
                    f_unit(g, c, w2c, ps_f)

            # rep teardown (LIFO per side)
            psf_cm.__exit__(None, None, None)
            w2s_cm.__exit__(None, None, None)
            po_cm.__exit__(None, None, None)
            ph3_cm.__exit__(None, None, None)
            pst_cm.__exit__(None, None, None)
            psq_cm.__exit__(None, None, None)
            pxq_cm.__exit__(None, None, None)
            ph2_cm.__exit__(None, None, None)
            pn_cm.__exit__(None, None, None)
            pe_cm.__exit__(None, None, None)
            ot_cm.__exit__(None, None, None)
            psc_cm.__exit__(None, None, None)
            wco_cm.__exit__(None, None, None)
            we_cm.__exit__(None, None, None)

    nc.compile()
    return nc


# ---------------- host side ----------------

def host_prep(x, Wq, Wk, Wv, Wo, bo, W1, b1, W2, b2, g1, be1, g2, be2):
    D = x.shape[2]
    H = Wq.shape[0]
    FF = W1.shape[1]
    NCF = FF // P
    f32 = np.float32

    mu = x.mean(-1, keepdims=True)
    var = ((x - mu) ** 2).mean(-1, keepdims=True)
    h1 = ((x - mu) / np.sqrt(var + EPS) * g1 + be1).astype(f32)

    shared = dict(
        wq=np.ascontiguousarray(
            WSCALE * Wq.transpose(1, 0, 2).reshape(D, -1)).astype(F8),
        wk=np.ascontiguousarray(
            WSCALE * Wk.transpose(1, 0, 2).reshape(D, -1)).astype(F8),
        wv=np.ascontiguousarray(
            WSCALE * Wv.transpose(1, 0, 2).reshape(D, -1)).astype(F8),
        wo=np.ascontiguousarray(Wo).astype(BF),
        w1=np.ascontiguousarray(g2[:, None] * W1).astype(BF),
        w2=np.ascontiguousarray(W2).astype(BF),
        b1c=np.ascontiguousarray((b1 + be2 @ W1).astype(f32).reshape(NCF, P).T),
    )
    per_core = []
    for c in range(8):
        b, p = c // 2, c % 2
        r = np.arange(P)[:, None]
        j = np.arange(256)[None, :]
        qoff = np.where(j < P, 2 * j + p, 256 + 2 * (j - P) + p)
        m4 = np.zeros((P, 4, 256), f32)
        for t in range(4):
            m4[:, t, :] = (P * t + r <= qoff)
        m = m4.reshape(P, 2, 2, 256)  # key-block-quad (pair, dk) layout
        per_core.append(dict(
            h1t=np.ascontiguousarray(h1[b].T).astype(F8),
            h1qt=np.ascontiguousarray(h1[b, p::2, :].T).astype(F8),
            xqt=np.ascontiguousarray(
                x[b, p::2, :].T + bo[:, None] + b2[:, None]).astype(f32),
            maskc=m.astype(BF),
            **shared,
        ))
    return per_core


def assemble(outts, B, T, D):
    out = np.zeros((B, T, D), np.float32)
    for c in range(8):
        b, p = c // 2, c % 2
        out[b, p::2, :] = outts[c].T
    return out


# ======================== top-level kernel entry ========================

_CACHE = {}


def _get_program():
    if "nc" not in _CACHE:
        _CACHE["nc"] = build_program(1024, 2048, 16, 64, 4096)
    return _CACHE["nc"]


def kernel(**inputs):
    """Full transformer block on 8 TRN2 NeuronCores.

    Takes the full unsharded inputs (as produced by setup_inputs()), shards
    (batch x query-parity) across 8 cores, runs the Bass SPMD kernel, and
    reassembles the full [4, 2048, 1024] float32 output.
    """
    from concourse.bass_utils import run_bass_kernel_spmd

    np_inputs = {k: np.asarray(v, np.float32) for k, v in inputs.items()}
    B, T, D = np_inputs["x"].shape
    nc = _get_program()
    per_core = host_prep(**np_inputs)
    res = run_bass_kernel_spmd(nc, per_core, list(range(8)))
    outts = [res.results[c]["outt"] for c in range(8)]
    return assemble(outts, B, T, D)


# revision 26
# speedup vs baseline: 2.0838x; 1.4772x over previous
"""Parametrized Bass/Tile transformer-block kernel for TRN2, 8-core SPMD.

Sharding: core c -> (batch b=c//2, query parity p=c%2). Each core computes the
output rows for tokens {2t+p} of its batch element. Fully uniform program, no
collectives; causal structure is uniform because local q-block i attends
exactly 2i+2 key blocks on every core (parity handled by a fixed mask).

Layout convention: "transposed" activations [feature, token] so matmuls chain
without transposes; per-token LN2 stats via ones-matmul column sums; softmax
denominator via a fused ones-column in the AV matmul.

Schedule: the block is emitted as interleaved "units" so the Tile scheduler
can fill the PE during the scalar-engine-bound (exp) attention phase.
Attention superblocks mp=0,1 interleave with the tail of the K/V projections;
mp=2,3 interleave with Wo/LN2/FFN1 for the first query group; the second
query group plus FFN2 run straight-line at the end (PE-bound, no scalar
pressure).  Wo/W1/W2 are streamed in small column tiles so all long-lived
tensors fit in SBUF simultaneously (the pool allocator is a strict per-side
LIFO stack, so lifetimes are arranged stack-fashion).
"""
import sys
for _p in ("/opt/trn_rl_repo", "/root/.axon_site/_ro/trn_rl_repo"):
    if _p not in sys.path:
        sys.path.append(_p)

from collections import deque
from contextlib import ExitStack

import numpy as np
import ml_dtypes

import concourse.bass as bass
import concourse.mybir as mybir
import concourse.tile as tile
from concourse import bacc
from concourse.bass import ts, ds

AF = mybir.ActivationFunctionType
DT = mybir.dt
BF = ml_dtypes.bfloat16
F8 = mybir.dt.np(mybir.dt.float8e4)
DR = mybir.MatmulPerfMode.DoubleRow
P = 128
EPS = 1e-5
WSCALE = 32.0   # fp8 weight pre-scale (host) undone at psum->SBUF copy


def build_program(D, T, H, DK, FF, reps=1, pipe=True, tune=True):
    Tq = T // 2
    NCD = D // P            # feature chunks
    NCF = FF // P           # hidden chunks
    NHP = H // 2            # head pairs
    NQG = Tq // 512         # 512-token groups over own queries
    NM = Tq // 256          # attention q superblocks (256 wide)
    NKB = T // P            # key blocks
    WV = min(512, D)        # V-projection column group width
    NVG = D // WV
    HPG = WV // DK          # heads per V col group
    assert D == H * DK and Tq % 512 == 0 and NHP == NCD

    nc = bacc.Bacc(None, target_bir_lowering=False)

    NCP = NCD // 2          # fp8 DoubleRow chunk pairs

    # ---- DRAM I/O ----
    h1t_d = nc.dram_tensor("h1t", [D, T], DT.float8e4, kind="ExternalInput")
    h1qt_d = nc.dram_tensor("h1qt", [D, Tq], DT.float8e4, kind="ExternalInput")
    xqt_d = nc.dram_tensor("xqt", [D, Tq], DT.float32, kind="ExternalInput")
    wq_d = nc.dram_tensor("wq", [D, D], DT.float8e4, kind="ExternalInput")
    wk_d = nc.dram_tensor("wk", [D, D], DT.float8e4, kind="ExternalInput")
    wv_d = nc.dram_tensor("wv", [D, D], DT.float8e4, kind="ExternalInput")
    wo_d = nc.dram_tensor("wo", [D, D], DT.bfloat16, kind="ExternalInput")
    w1_d = nc.dram_tensor("w1", [D, FF], DT.bfloat16, kind="ExternalInput")
    w2_d = nc.dram_tensor("w2", [FF, D], DT.bfloat16, kind="ExternalInput")
    b1c_d = nc.dram_tensor("b1c", [P, NCF], DT.float32, kind="ExternalInput")
    mask_d = nc.dram_tensor("maskc", [P, 2, 2, 256], DT.bfloat16,
                            kind="ExternalInput")
    outt_d = nc.dram_tensor("outt", [D, Tq], DT.float32, kind="ExternalOutput")

    h1t_r = h1t_d[:].rearrange("(c p) t -> p c t", p=P)
    h1qt_r = h1qt_d[:].rearrange("(c p) t -> p c t", p=P)
    xqt_r = xqt_d[:].rearrange("(c p) t -> p c t", p=P)
    wq_r = wq_d[:].rearrange("(c p) n -> p c n", p=P)
    wk_r = wk_d[:].rearrange("(c p) n -> p c n", p=P)
    wv_r = wv_d[:].rearrange("(c p) n -> p c n", p=P)
    wo_r = wo_d[:].rearrange("(c p) n -> p c n", p=P)
    w1_r = w1_d[:].rearrange("(c p) n -> p c n", p=P)
    w2_r = w2_d[:].rearrange("(c p) n -> p c n", p=P)
    outt_r = outt_d[:].rearrange("(c p) t -> p c t", p=P)

    with tile.TileContext(nc) as tc, ExitStack() as top:
        constp = top.enter_context(tc.tile_pool(name="const", bufs=1))
        mask_sb = constp.tile([P, 2, 2, 256], DT.bfloat16)
        nc.sync.dma_start(mask_sb[:], mask_d[:])
        b1c_sb = constp.tile([P, NCF], DT.float32)
        nc.sync.dma_start(b1c_sb[:], b1c_d[:])
        ones_bf = constp.tile([P, 1], DT.bfloat16)
        nc.vector.memset(ones_bf[:], 1.0)
        ones_f = constp.tile([1, P], DT.float32)
        nc.vector.memset(ones_f[:], 1.0)
        ones_r = constp.tile([1, P], DT.float32r)
        with nc.allow_low_precision(reason="f32r ones"):
            nc.vector.tensor_copy(ones_r[:], ones_f[:])
        zero_b = constp.tile([P, 1], DT.float32)
        nc.vector.memset(zero_b[:], 0.0)
        eps_b = constp.tile([1, 1], DT.float32)
        nc.vector.memset(eps_b[:], EPS)

        for _rep in range(reps):
            # ------- left stack (LIFO): w_e | w_co | qkv | a2 | w_a | h1q -------
            we_cm = tc.tile_pool(name="w_e", bufs=3, side="left")
            w_e = we_cm.__enter__()
            wco_cm = tc.tile_pool(name="w_co", bufs=2, side="left")
            w_co = wco_cm.__enter__()

            qkv_cm = tc.tile_pool(name="p_qkv", bufs=1, side="left")
            p_qkv = qkv_cm.__enter__()
            qt_sb = p_qkv.tile([P, NHP, Tq], DT.bfloat16)
            kt_sb = p_qkv.tile([P, NHP, T], DT.bfloat16)
            v4_sb = p_qkv.tile([P, NKB, H, 65], DT.float8e4)

            a2_cm = tc.tile_pool(name="p_a2", bufs=1, side="left")
            p_a2 = a2_cm.__enter__()
            h1t_sb = p_a2.tile([P, NCD, T], DT.float8e4)
            wv_sb = p_a2.tile([P, NCD, D], DT.float8e4)

            wa_cm = tc.tile_pool(name="w_a", bufs=2, side="left")
            w_a = wa_cm.__enter__()

            h1q_cm = tc.tile_pool(name="p_h1q", bufs=1, side="left")
            p_h1q = h1q_cm.__enter__()
            h1qt_sb = p_h1q.tile([P, NCD, Tq], DT.float8e4)

            # ------- PSUM stack (LIFO): ps_c | ps_s | ps_o -------
            psc_cm = tc.tile_pool(name="ps_c", bufs=2, space="PSUM")
            ps_c = psc_cm.__enter__()
            pss_cm = tc.tile_pool(name="ps_s", bufs=2, space="PSUM")
            ps_s = pss_cm.__enter__()
            pso_cm = tc.tile_pool(name="ps_o", bufs=2, space="PSUM")
            ps_o = pso_cm.__enter__()

            # ---------------- input DMAs ----------------
            for c in range(NCD):
                nc.gpsimd.dma_start(h1qt_sb[:, c], h1qt_r[:, c])
            for c in range(NCD):
                nc.sync.dma_start(wv_sb[:, c], wv_r[:, c])
            for g in range(T // 512):
                for c in range(NCD):
                    eng = nc.scalar if (c % 2 == 0) else nc.gpsimd
                    eng.dma_start(h1t_sb[:, c, ds(512 * g, 512)],
                                  h1t_r[:, c, ds(512 * g, 512)])
            # softmax-denominator ones-column
            nc.vector.memset(v4_sb[:, :, :, 64:65], 1.0)

            # per-group tiles, created lazily at phase boundaries
            x1_t, xh_t, g1_t = {}, {}, {}

            # ---------------- unit emitters ----------------
            def q_unit(g, hp):
                wq_t = w_a.tile([P, NCD, P], DT.float8e4, tag="wq", name="wq_t")
                nc.sync.dma_start(wq_t[:], wq_r[:, :, ds(P * hp, P)])
                ps_q = ps_c.tile([P, 512], DT.float32, tag="chain", name="ps_q")
                for cp in range(NCP):
                    nc.tensor.matmul(
                        ps_q[:], wq_t[:, 2 * cp:2 * cp + 2, :],
                        h1qt_sb[:, 2 * cp:2 * cp + 2, ds(512 * g, 512)],
                        start=(cp == 0), stop=(cp == NCP - 1), perf_mode=DR)
                nc.vector.tensor_scalar_mul(
                    qt_sb[:, hp, ds(512 * g, 512)], ps_q[:], 1.0 / WSCALE)

            def k_unit(g, hp):
                wk_t = w_a.tile([P, NCD, P], DT.float8e4, tag="wk", name="wk_t")
                nc.sync.dma_start(wk_t[:], wk_r[:, :, ds(P * hp, P)])
                ps_k = ps_c.tile([P, 512], DT.float32, tag="chain", name="ps_k")
                for cp in range(NCP):
                    nc.tensor.matmul(
                        ps_k[:], wk_t[:, 2 * cp:2 * cp + 2, :],
                        h1t_sb[:, 2 * cp:2 * cp + 2, ds(512 * g, 512)],
                        start=(cp == 0), stop=(cp == NCP - 1), perf_mode=DR)
                nc.vector.tensor_scalar_mul(
                    kt_sb[:, hp, ds(512 * g, 512)], ps_k[:], 1.0 / WSCALE)

            def v_unit(tb, g2):
                ps_v = ps_c.tile([P, WV], DT.float32, tag="chain", name="ps_v")
                for cp in range(NCP):
                    nc.tensor.matmul(
                        ps_v[:], h1t_sb[:, 2 * cp:2 * cp + 2, ds(P * tb, P)],
                        wv_sb[:, 2 * cp:2 * cp + 2, ds(WV * g2, WV)],
                        start=(cp == 0), stop=(cp == NCP - 1), perf_mode=DR)
                for hh in range(HPG):
                    h = HPG * g2 + hh
                    nc.vector.tensor_scalar_mul(
                        v4_sb[:, tb, h, 0:64], ps_v[:, ds(64 * hh, 64)],
                        1.0 / WSCALE)

            def emit_av(po, hp, K0, exs, nkv):
                for hi in range(2):
                    h = 2 * hp + hi
                    for dk in range(4):
                        kb = K0 + dk
                        nc.tensor.matmul(
                            po[hi][0:65, :],
                            v4_sb[:, kb, h, 0:65],
                            exs[hi][:, dk // 2, dk % 2],
                            start=(kb == 0), stop=(kb == nkv - 1))

            def attn_unit(mp, hp):
                # AV emission lags scores/exp by one key-block quad so the
                # PE always has independent scores work ahead of each
                # exp-blocked AV.
                nkv = 4 * mp + 4
                po = [ps_o.tile([P, 256], DT.float32, tag="po", name=f"po{_i}")
                      for _i in range(2)]
                lag = None
                for Kq in range(mp + 1):
                    K0 = 4 * Kq
                    exs = []
                    for hi in range(2):
                        pb = 64 * hi
                        ps = ps_s.tile([P, 1024], DT.float32, tag="ps", name="ps")
                        for dk in range(4):
                            nc.tensor.matmul(
                                ps[:, ds(256 * dk, 256)],
                                kt_sb[pb:pb + 64, hp, ds(P * (K0 + dk), P)],
                                qt_sb[pb:pb + 64, hp, ds(256 * mp, 256)],
                                start=True, stop=True,
                                tile_position=(pb, 0))
                        ex = p_exp.tile([P, 2, 2, 256], DT.bfloat16, tag="ex",
                                        name="ex")
                        nc.scalar.activation(ex[:], ps[:], AF.Exp, bias=zero_b[:],
                                             scale=float(DK) ** -0.5)
                        if Kq == mp:
                            nc.vector.tensor_mul(ex[:], ex[:], mask_sb[:])
                        exs.append(ex)
                    if lag is not None:
                        emit_av(po, hp, lag[0], lag[1], nkv)
                    lag = (K0, exs)
                emit_av(po, hp, lag[0], lag[1], nkv)
                for hi in range(2):
                    rec = p_nrm.tile([1, 256], DT.float32r, tag="rec", name="rec")
                    with nc.allow_low_precision(reason="f32r broadcast operand"):
                        nc.vector.reciprocal(rec[:], po[hi][64:65, :])
                    pbc = ps_c.tile([P, 256], DT.float32, tag="chain", name="pbc")
                    nc.tensor.matmul(
                        pbc[0:64, :], ones_r[:, 0:64], rec[:],
                        start=True, stop=True)
                    rb = p_nrm.tile([64, 256], DT.float32, tag="rb", name="rb")
                    nc.vector.tensor_copy(rb[:], pbc[0:64, :])
                    nc.vector.tensor_mul(
                        ot_sb[64 * hi:64 * hi + 64, hp, ds(256 * mp, 256)],
                        po[hi][0:64, :], rb[:])

            def c_unit(g, c):
                wo_t = w_co.tile([P, NCD, P], DT.bfloat16, tag="wo", name="wo_t")
                nc.sync.dma_start(wo_t[:], wo_r[:, :, ds(P * c, P)])
                ps_x = ps_c.tile([P, 512], DT.float32, tag="chain", name="ps_x")
                for k in range(NCD):
                    nc.tensor.matmul(
                        ps_x[:], wo_t[:, k],
                        ot_sb[:, k, ds(512 * g, 512)],
                        start=(k == 0), stop=(k == NCD - 1))
                xq_t = p_xq.tile([P, 512], DT.float32, tag="xq", name="xq_t")
                nc.sync.dma_start(xq_t[:], xqt_r[:, c, ds(512 * g, 512)])
                nc.vector.tensor_add(x1_t[g][:, c, :], ps_x[:], xq_t[:])

            def d_unit(g):
                x1g = x1_t[g]
                ps1 = ps_c.tile([P, 512], DT.float32, tag="chain", name="ps1")
                for c in range(NCD):
                    nc.tensor.matmul(ps1[0:1, :], ones_bf[:], x1g[:, c, :],
                                     start=(c == 0), stop=(c == NCD - 1))
                ps2 = ps_c.tile([P, 512], DT.float32, tag="chain", name="ps2")
                for c in range(NCD):
                    sq = p_sq.tile([P, 512], DT.bfloat16, tag="sq", name="sq")
                    nc.vector.tensor_mul(sq[:], x1g[:, c, :], x1g[:, c, :])
                    nc.tensor.matmul(ps2[0:1, :], ones_bf[:], sq[:],
                                     start=(c == 0), stop=(c == NCD - 1))
                mu = p_st.tile([1, 512], DT.float32, tag="mu", name="mu")
                nc.vector.tensor_scalar_mul(mu[:], ps1[0:1, :], 1.0 / D)
                msq = p_st.tile([1, 512], DT.float32, tag="msq", name="msq")
                nc.vector.tensor_mul(msq[:], mu[:], mu[:])
                nc.vector.scalar_tensor_tensor(
                    msq[:], ps2[0:1, :], 1.0 / D, msq[:],
                    mybir.AluOpType.mult, mybir.AluOpType.subtract)
                sd = p_st.tile([1, 512], DT.float32, tag="sd", name="sd")
                nc.scalar.activation(sd[:], msq[:], AF.Sqrt, bias=eps_b[:])
                r2 = p_st.tile([1, 512], DT.float32r, tag="r2", name="r2")
                with nc.allow_low_precision(reason="f32r broadcast operand"):
                    nc.vector.reciprocal(r2[:], sd[:])
                m2r = p_st.tile([1, 512], DT.float32r, tag="m2r", name="m2r")
                with nc.allow_low_precision(reason="f32r broadcast operand"):
                    nc.vector.tensor_mul(m2r[:], mu[:], r2[:])
                pb1 = ps_c.tile([P, 512], DT.float32, tag="chain", name="pb1")
                nc.tensor.matmul(pb1[:], ones_r[:], r2[:], start=True, stop=True)
                r2b = p_sq.tile([P, 512], DT.float32, tag="r2b", name="r2b",
                                bufs=1)
                nc.vector.tensor_copy(r2b[:], pb1[:])
                pb2 = ps_c.tile([P, 512], DT.float32, tag="chain", name="pb2")
                nc.tensor.matmul(pb2[:], ones_r[:], m2r[:], start=True, stop=True)
                m2rb = p_sq.tile([P, 512], DT.float32, tag="m2rb", name="m2rb",
                                 bufs=1)
                nc.vector.tensor_copy(m2rb[:], pb2[:])
                for c in range(NCD):
                    tmp = p_sq.tile([P, 512], DT.float32, tag="tmp", name="tmp")
                    nc.vector.tensor_mul(tmp[:], x1g[:, c, :], r2b[:])
                    nc.vector.tensor_sub(xh_t[g][:, c, :], tmp[:], m2rb[:])

            def e_unit(g, m):
                w1_t = w_e.tile([P, NCD, P], DT.bfloat16, tag="w1", name="w1_t")
                nc.gpsimd.dma_start(w1_t[:], w1_r[:, :, ds(P * m, P)])
                ps_f = ps_c.tile([P, 512], DT.float32, tag="chain", name="ps_f")
                for c in range(NCD):
                    nc.tensor.matmul(
                        ps_f[:], w1_t[:, c], xh_t[g][:, c, :],
                        start=(c == 0), stop=(c == NCD - 1))
                nc.scalar.activation(
                    g1_t[g][:, m, :], ps_f[:], AF.Gelu, bias=b1c_sb[:, ds(m, 1)])

            def f_unit(g, c, w2c, ps_f):
                pf = ps_f.tile([P, 512], DT.float32, tag="chain", name="pf")
                for hh in range(NCF):
                    nc.tensor.matmul(
                        pf[:], w2c[:, hh], g1_t[g][:, hh, :],
                        start=(hh == 0), stop=(hh == NCF - 1))
                out_t = p_out.tile([P, 512], DT.float32, tag="out", name="out_t")
                nc.vector.tensor_add(out_t[:], pf[:], x1_t[g][:, c, :])
                nc.sync.dma_start(outt_r[:, c, ds(512 * g, 512)], out_t[:])

            # ---------------- emission schedule ----------------
            filler = deque()

            def drain(n):
                for _ in range(n):
                    if not filler:
                        return
                    filler.popleft()()

            # Phase 0a: q projections (the only h1qt consumers).
            for g in range(NQG):
                for hp in range(NHP):
                    q_unit(g, hp)
            h1q_cm.__exit__(None, None, None)

            # Phase 0b: minimum K/V work before attention mp=0 can start.
            for hp in range(NHP):
                k_unit(0, hp)
            for tb in range(4):
                for g2 in range(NVG):
                    v_unit(tb, g2)

            # Fillers for phase 1 (rest of K/V, dependency order).
            for hp in range(NHP):
                filler.append(lambda hp=hp: k_unit(1, hp))
            for tb in range(4, 8):
                for g2 in range(NVG):
                    filler.append(lambda tb=tb, g2=g2: v_unit(tb, g2))
            for hp in range(NHP):
                filler.append(lambda hp=hp: k_unit(2, hp))
            for tb in range(8, 12):
                for g2 in range(NVG):
                    filler.append(lambda tb=tb, g2=g2: v_unit(tb, g2))
            for hp in range(NHP):
                filler.append(lambda hp=hp: k_unit(3, hp))
            for tb in range(12, 16):
                for g2 in range(NVG):
                    filler.append(lambda tb=tb, g2=g2: v_unit(tb, g2))

            # attention output + exp scratch pools (right side)
            ot_cm = tc.tile_pool(name="p_ot", bufs=1, side="right")
            p_ot = ot_cm.__enter__()
            ot_sb = p_ot.tile([P, NCD, Tq], DT.bfloat16)
            pe_cm = tc.tile_pool(name="p_exp", bufs=5, side="right")
            p_exp = pe_cm.__enter__()
            pn_cm = tc.tile_pool(name="p_nrm", bufs=2, side="right")
            p_nrm = pn_cm.__enter__()

            # Phase 1: attention mp=0,1 interleaved with K/V tail.
            for mp in range(2):
                for hp in range(NHP):
                    attn_unit(mp, hp)
                    drain(2 if mp == 0 else 3)
            while filler:
                filler.popleft()()
            wa_cm.__exit__(None, None, None)
            a2_cm.__exit__(None, None, None)

            # phase-2 pools (right side, live to rep end)
            ph2_cm = tc.tile_pool(name="p_ph2", bufs=1, side="right")
            p_ph2 = ph2_cm.__enter__()
            x1_t[0] = p_ph2.tile([P, NCD, 512], DT.bfloat16, name="x1_0")
            xh_t[0] = p_ph2.tile([P, NCD, 512], DT.bfloat16, name="xh_0")
            g1_t[0] = p_ph2.tile([P, NCF, 512], DT.bfloat16, name="g1_0")
            pxq_cm = tc.tile_pool(name="p_xq", bufs=3, side="right")
            p_xq = pxq_cm.__enter__()
            psq_cm = tc.tile_pool(name="p_sq", bufs=2, side="right")
            p_sq = psq_cm.__enter__()
            pst_cm = tc.tile_pool(name="p_st", bufs=1, side="right")
            p_st = pst_cm.__enter__()
            po_cm = tc.tile_pool(name="p_out", bufs=4, side="right")
            p_out = po_cm.__enter__()

            # w2 column-chunk stream pool for group-0 FFN2 (phase 2)
            w2a_cm = tc.tile_pool(name="w2a", bufs=2, side="left")
            w2a = w2a_cm.__enter__()

            def f0_unit(c):
                w2c = w2a.tile([P, NCF, P], DT.bfloat16, tag="w2c", name="w2c")
                nc.gpsimd.dma_start(w2c[:], w2_r[:, :, ds(P * c, P)])
                f_unit(0, c, w2c, ps_c)

            # Fillers for phase 2: Wo+LN2+FFN1+FFN2 for query group 0.
            for c in range(NCD):
                filler.append(lambda c=c: c_unit(0, c))
            filler.append(lambda: d_unit(0))
            for m in range(NCF):
                filler.append(lambda m=m: e_unit(0, m))
            for c in range(NCD):
                filler.append(lambda c=c: f0_unit(c))

            # Phase 2: attention mp=2,3 interleaved with group-0 FFN work.
            for mp in range(2, 4):
                for hp in range(NHP):
                    attn_unit(mp, hp)
                    drain(3)
            while filler:
                filler.popleft()()

            # attention done: free scores/AV PSUM and q/k/v SBUF.
            w2a_cm.__exit__(None, None, None)
            pso_cm.__exit__(None, None, None)
            pss_cm.__exit__(None, None, None)
            qkv_cm.__exit__(None, None, None)

            # phase-3 pools
            ph3_cm = tc.tile_pool(name="p_ph3", bufs=1, side="right")
            p_ph3 = ph3_cm.__enter__()
            x1_t[1] = p_ph3.tile([P, NCD, 512], DT.bfloat16, name="x1_1")
            xh_t[1] = p_ph3.tile([P, NCD, 512], DT.bfloat16, name="xh_1")
            g1_t[1] = p_ph3.tile([P, NCF, 512], DT.bfloat16, name="g1_1")
            w2s_cm = tc.tile_pool(name="w2s", bufs=2, side="left")
            w2s = w2s_cm.__enter__()
            psf_cm = tc.tile_pool(name="ps_f", bufs=3, space="PSUM")
            ps_f = psf_cm.__enter__()

            # Phase 3: group-1 Wo/LN2/FFN1, then its FFN2 with w2 streamed
            # per output column chunk.
            for c in range(NCD):
                c_unit(1, c)
            d_unit(1)
            for m in range(NCF):
                e_unit(1, m)
            for c in range(NCD):
                w2c = w2s.tile([P, NCF, P], DT.bfloat16, tag="w2c", name="w2c")
                nc.gpsimd.dma_start(w2c[:], w2_r[:, :, ds(P * c, P)])
                f_unit(1, c, w2c, ps_f)

            # rep teardown (LIFO per side)
            psf_cm.__exit__(None, None, None)
            w2s_cm.__exit__(None, None, None)
            ph3_cm.__exit__(None, None, None)
            po_cm.__exit__(None, None, None)
            pst_cm.__exit__(None, None, None)
            psq_cm.__exit__(None, None, None)
            pxq_cm.__exit__(None, None, None)
            ph2_cm.__exit__(None, None, None)
            pn_cm.__exit__(None, None, None)
            pe_cm.__exit__(None, None, None)
            ot_cm.__exit__(None, None, None)
            psc_cm.__exit__(None, None, None)
            wco_cm.__exit__(None, None, None)
            we_cm.__exit__(None, None, None)

    nc.compile()
    return nc


# ---------------- host side ----------------

def host_prep(x, Wq, Wk, Wv, Wo, bo, W1, b1, W2, b2, g1, be1, g2, be2):
    D = x.shape[2]
    H = Wq.shape[0]
    FF = W1.shape[1]
    NCF = FF // P
    f32 = np.float32

    mu = x.mean(-1, keepdims=True)
    var = ((x - mu) ** 2).mean(-1, keepdims=True)
    h1 = ((x - mu) / np.sqrt(var + EPS) * g1 + be1).astype(f32)

    shared = dict(
        wq=np.ascontiguousarray(
            WSCALE * Wq.transpose(1, 0, 2).reshape(D, -1)).astype(F8),
        wk=np.ascontiguousarray(
            WSCALE * Wk.transpose(1, 0, 2).reshape(D, -1)).astype(F8),
        wv=np.ascontiguousarray(
            WSCALE * Wv.transpose(1, 0, 2).reshape(D, -1)).astype(F8),
        wo=np.ascontiguousarray(Wo).astype(BF),
        w1=np.ascontiguousarray(g2[:, None] * W1).astype(BF),
        w2=np.ascontiguousarray(W2).astype(BF),
        b1c=np.ascontiguousarray((b1 + be2 @ W1).astype(f32).reshape(NCF, P).T),
    )
    per_core = []
    for c in range(8):
        b, p = c // 2, c % 2
        r = np.arange(P)[:, None]
        j = np.arange(256)[None, :]
        qoff = np.where(j < P, 2 * j + p, 256 + 2 * (j - P) + p)
        m4 = np.zeros((P, 4, 256), f32)
        for t in range(4):
            m4[:, t, :] = (P * t + r <= qoff)
        m = m4.reshape(P, 2, 2, 256)  # key-block-quad (pair, dk) layout
        per_core.append(dict(
            h1t=np.ascontiguousarray(h1[b].T).astype(F8),
            h1qt=np.ascontiguousarray(h1[b, p::2, :].T).astype(F8),
            xqt=np.ascontiguousarray(
                x[b, p::2, :].T + bo[:, None] + b2[:, None]).astype(f32),
            maskc=m.astype(BF),
            **shared,
        ))
    return per_core


def assemble(outts, B, T, D):
    out = np.zeros((B, T, D), np.float32)
    for c in range(8):
        b, p = c // 2, c % 2
        out[b, p::2, :] = outts[c].T
    return out


# ======================== top-level kernel entry ========================

_CACHE = {}


def _get_program():
    if "nc" not in _CACHE:
        _CACHE["nc"] = build_program(1024, 2048, 16, 64, 4096)
    return _CACHE["nc"]


def kernel(**inputs):
    """Full transformer block on 8 TRN2 NeuronCores.

    Takes the full unsharded inputs (as produced by setup_inputs()), shards
    (batch x query-parity) across 8 cores, runs the Bass SPMD kernel, and
    reassembles the full [4, 2048, 1024] float32 output.
    """
    from concourse.bass_utils import run_bass_kernel_spmd

    np_inputs = {k: np.asarray(v, np.float32) for k, v in inputs.items()}
    B, T, D = np_inputs["x"].shape
    nc = _get_program()
    per_core = host_prep(**np_inputs)
    res = run_bass_kernel_spmd(nc, per_core, list(range(8)))
    outts = [res.results[c]["outt"] for c in range(8)]
    return assemble(outts, B, T, D)
